# revision 1
# baseline (speedup 1.0000x reference)
"""Trainium2 Bass kernel for DeepME edge-MLP (gnn_message_passing).

Contract: kernel(**inputs) takes FULL unsharded inputs (as produced by the
reference setup_inputs()) and returns the FULL [E, 1] float32 output.

Strategy: data-parallel over the edge dimension across 8 NeuronCores.
Embedding table and (small) MLP weights are replicated per core.

Per-core device program (SPMD, one Bass program):
  - edge indices / types resident in SBUF, partition-major layout
  - per 512-edge tile:
      gather src/dst embedding rows (indirect DMA, 128 rows per descriptor)
      PE-transpose into feature-on-partition layout
      branch matmuls (192->64 x3 on diff/diff^2/src*dst, 192->192 x2)
      relu+bias fused into PSUM eviction on the scalar engine
      PE-transpose down to edge-on-partition layout, LayerNorm via bn_stats
        (gamma/beta folded into the next layer's weights host-side)
      PE-transpose up, fusion MLP 576->192 (LN) ->192 ->3
      output probability = 1 / sum_j exp(l_j - l_{edge_type})
"""

import numpy as np

# ---------------------------------------------------------------------------
# problem constants (hardcoded per the harness contract)
E_TOTAL = 300000
N_NODES = 300000
H = 192
H3 = 64
NCORES = 8
P = 128
CH = 4                 # 128-edge chunks per tile
TILE = P * CH          # 512 edges per tile
E_PC = E_TOTAL // NCORES          # 37500 edges per core
NTILES = (E_PC + TILE - 1) // TILE  # 74
E_PAD = NTILES * TILE               # 37888
LN_EPS = 1e-5

_PROG_CACHE = {}


def _build_program(n_tiles, n_nodes, mmdt="f32", repeat=1):
    """Build the SPMD Bass program. Returns the Bass object."""
    from contextlib import ExitStack

    import concourse.bass as bass
    import concourse.bacc as bacc
    import concourse.tile as tile
    import concourse.mybir as mybir

    dt = mybir.dt
    f32 = dt.float32
    i32 = dt.int32
    MMDT = {"f32": f32, "f32r": dt.float32r, "bf16": dt.bfloat16,
            "f32rb": dt.float32r}[mmdt]
    EMBDT = dt.bfloat16 if mmdt in ("bf16", "f32rb") else f32
    TRDT = dt.bfloat16 if mmdt == "bf16" else f32   # transpose-path dtype
    GDT = EMBDT if EMBDT != f32 else TRDT           # gather-transpose dtype

    def rd(ap):
        # read view of an MMDT tile for elementwise engines
        return ap.bitcast(f32) if mmdt in ("f32r", "f32rb") else ap
    AF = mybir.ActivationFunctionType
    OP = mybir.AluOpType
    AX = mybir.AxisListType

    nedge_cols = n_tiles * CH

    nc = bacc.Bacc(trn_type="TRN2", target_bir_lowering=False, debug=False,
                   num_devices=NCORES)

    # ----- DRAM parameters ------------------------------------------------
    emb = nc.dram_tensor("emb", [n_nodes, H], EMBDT, kind="ExternalInput").ap()
    sidx_d = nc.dram_tensor("sidx", [P, nedge_cols], i32, kind="ExternalInput").ap()
    didx_d = nc.dram_tensor("didx", [P, nedge_cols], i32, kind="ExternalInput").ap()
    etf_d = nc.dram_tensor("etf", [P, nedge_cols], f32, kind="ExternalInput").ap()
    w1_d = nc.dram_tensor("w1", [H, H3], f32, kind="ExternalInput").ap()
    w2_d = nc.dram_tensor("w2", [H, H3], f32, kind="ExternalInput").ap()
    w3_d = nc.dram_tensor("w3", [H, H3], f32, kind="ExternalInput").ap()
    ws_d = nc.dram_tensor("ws", [H, H], f32, kind="ExternalInput").ap()
    wd_d = nc.dram_tensor("wd", [H, H], f32, kind="ExternalInput").ap()
    wf1_d = nc.dram_tensor("wf1", [3 * H, H], f32, kind="ExternalInput").ap()
    wf2_d = nc.dram_tensor("wf2", [H, H], f32, kind="ExternalInput").ap()
    wf3_d = nc.dram_tensor("wf3", [H, 4], f32, kind="ExternalInput").ap()
    nc1_d = nc.dram_tensor("nc1", [5, H], f32, kind="ExternalInput").ap()
    nc1f_d = nc.dram_tensor("nc1f", [1, H], f32, kind="ExternalInput").ap()
    # packed per-partition bias columns (see kernel() for layout)
    bias_d = nc.dram_tensor("biascol", [P, 12], f32, kind="ExternalInput").ap()
    # consts: identity(128x128) | iota3 (12) | c4N (20)
    cst_d = nc.dram_tensor("consts", [P, P + 12 + 20], f32, kind="ExternalInput").ap()
    out_d = nc.dram_tensor("out", [P, nedge_cols], f32, kind="ExternalOutput").ap()

    def mm(out, lhsT, rhs, start, stop=True):
        nc.tensor.matmul(out=out, lhsT=lhsT, rhs=rhs, start=start, stop=stop)

    with tile.TileContext(nc) as tc, ExitStack() as ctx:
        cpool = ctx.enter_context(tc.tile_pool(name="const", bufs=1))
        sb = ctx.enter_context(tc.tile_pool(name="work", bufs=1))
        sb2 = ctx.enter_context(tc.tile_pool(name="work2", bufs=2))
        pp = ctx.enter_context(tc.tile_pool(name="psum", bufs=1, space="PSUM"))

        # ----- resident tiles (loaded once) -------------------------------
        sidx = cpool.tile([P, nedge_cols], i32)
        didx = cpool.tile([P, nedge_cols], i32)
        etf = cpool.tile([P, nedge_cols], f32)
        outp = cpool.tile([P, nedge_cols], f32)
        nc.sync.dma_start(sidx[:], sidx_d[:])
        nc.sync.dma_start(didx[:], didx_d[:])
        nc.sync.dma_start(etf[:], etf_d[:])

        def wload(shape, src_ap, name):
            t_ = cpool.tile(shape, MMDT, name=name)
            if mmdt == "f32":
                nc.sync.dma_start(t_[:], src_ap)
            else:
                stg = cpool.tile(shape, f32, name=f"{name}_stg")
                nc.sync.dma_start(stg[:], src_ap)
                nc.vector.tensor_copy(t_[:], stg[:])
            return t_

        w1h = wload([P, H3], w1_d[0:P, :], "w1h")
        w1l = wload([H - P, H3], w1_d[P:H, :], "w1l")
        w2h = wload([P, H3], w2_d[0:P, :], "w2h")
        w2l = wload([H - P, H3], w2_d[P:H, :], "w2l")
        w3h = wload([P, H3], w3_d[0:P, :], "w3h")
        w3l = wload([H - P, H3], w3_d[P:H, :], "w3l")
        wsh = wload([P, H], ws_d[0:P, :], "wsh")
        wsl = wload([H - P, H], ws_d[P:H, :], "wsl")
        wdh = wload([P, H], wd_d[0:P, :], "wdh")
        wdl = wload([H - P, H], wd_d[P:H, :], "wdl")
        wf2h = wload([P, H], wf2_d[0:P, :], "wf2h")
        wf2l = wload([H - P, H], wf2_d[P:H, :], "wf2l")
        wf3h = wload([P, 4], wf3_d[0:P, :], "wf3h")
        wf3l = wload([H - P, 4], wf3_d[P:H, :], "wf3l")
        # fusion weight: five K-chunks (rows of [sx | dx | b1 | b2 | b3])
        wf1t = [wload([P, H], wf1_d[k * P:(k + 1) * P, :], f"wf1t{k}")
                for k in range(4)]
        wf1e = wload([64, H], wf1_d[512:576, :], "wf1e")
        nc1w = wload([5, H], nc1_d[:, :], "nc1w")
        nc1fw = wload([1, H], nc1f_d[:, :], "nc1fw")

        bias = cpool.tile([P, 12], f32)
        nc.sync.dma_start(bias[:], bias_d[:])
        cst = cpool.tile([P, P + 12 + 20], f32)
        nc.sync.dma_start(cst[:], cst_d[:])
        ident = cst[:, 0:P]
        identb = None
        if TRDT != f32 or EMBDT != f32:
            identbt = cpool.tile([P, P], dt.bfloat16, name="identbt")
            nc.vector.tensor_copy(identbt[:], cst[:, 0:P])
            identb = identbt[:]
        iota3 = cst[:, P:P + 12].rearrange("p (c t) -> p c t", t=3)
        c4n = cst[:, P + 12:P + 32].rearrange("p (c b) -> p c b", b=5)

        def tp(out, in_):
            k = in_.partition_size()
            idn = identb if (identb is not None
                             and in_.dtype == dt.bfloat16) else ident
            nc.tensor.transpose(out=out, in_=in_, identity=idn[0:k, 0:k])

        # bias column layout (see kernel()):
        # 0: [b1;b2]  1: [b3;0]  2: bs_hi  3: bs_lo  4: bd_hi  5: bd_lo
        # 6: bf1_hi   7: bf1_lo  8: bf2_hi 9: bf2_lo 10: [bf3;0] 11: 4*eps
        def bcol(j, np_=P):
            return bias[0:np_, j:j + 1]

        # ----- two-phase pipelined tile loop ------------------------------
        # part1(t): gather -> transposes -> branch matmuls -> relu evict ->
        #           transpose-down -> LN stats+finalize
        # part2(t): LN apply -> transpose-up -> fusion MLP -> softmax-select
        # Emitting part1(t) before part2(t-1) lets the PE work on tile t's
        # gathers/branches while the vector engines finish tile t-1's LN.
        # PSUM tags: part1 {A,B,H,C,F,G}, part2 {D,E}.

        def part1(t):
            srcG = sb2.tile([P, CH, H], EMBDT, tag="srcG", name="srcG")
            dstG = sb2.tile([P, CH, H], EMBDT, tag="dstG", name="dstG")
            for c in range(CH):
                nc.gpsimd.indirect_dma_start(
                    out=srcG[:, c, :], out_offset=None, in_=emb[:, :],
                    in_offset=bass.IndirectOffsetOnAxis(
                        ap=sidx[:, t * CH + c: t * CH + c + 1], axis=0))
                nc.gpsimd.indirect_dma_start(
                    out=dstG[:, c, :], out_offset=None, in_=emb[:, :],
                    in_offset=bass.IndirectOffsetOnAxis(
                        ap=didx[:, t * CH + c: t * CH + c + 1], axis=0))

            # transpose to feature-major: srcT = [192, 512] as two tiles
            sTA = pp.tile([P, TILE], GDT, tag="psA", name="sTA")
            dTA = pp.tile([P, TILE], GDT, tag="psH", name="dTA")
            sTB = pp.tile([64, TILE], GDT, tag="psB", name="sTB")
            srcTA = sb.tile([P, TILE], MMDT, tag="srcTA", bufs=2, name="srcTA")
            srcTB = sb.tile([64, TILE], MMDT, tag="srcTB", bufs=2, name="srcTB")
            dstTA = sb.tile([P, TILE], MMDT, tag="dstTA", bufs=2, name="dstTA")
            dstTB = sb.tile([64, TILE], MMDT, tag="dstTB", bufs=2, name="dstTB")
            for c in range(CH):
                cs = slice(c * P, (c + 1) * P)
                tp(sTA[:, cs], srcG[:, c, 0:P])
                tp(sTB[:, cs], srcG[:, c, P:H])
                tp(dTA[:, cs], dstG[:, c, 0:P])
            nc.any.tensor_copy(srcTB[:], sTB[:])
            dTB = pp.tile([64, TILE], GDT, tag="psB", name="dTB")
            for c in range(CH):
                cs = slice(c * P, (c + 1) * P)
                tp(dTB[:, cs], dstG[:, c, P:H])
            nc.any.tensor_copy(srcTA[:], sTA[:])
            nc.any.tensor_copy(dstTA[:], dTA[:])
            nc.any.tensor_copy(dstTB[:], dTB[:])

            # elementwise: diff, prod, diff^2 (feature-major)
            difA = sb.tile([P, TILE], MMDT, tag="difA", bufs=2, name="difA")
            difB = sb.tile([64, TILE], MMDT, tag="difB", bufs=2, name="difB")
            prdA = sb.tile([P, TILE], MMDT, tag="prdA", bufs=2, name="prdA")
            prdB = sb.tile([64, TILE], MMDT, tag="prdB", bufs=2, name="prdB")
            sqA = sb.tile([P, TILE], MMDT, tag="sqA", bufs=2, name="sqA")
            sqB = sb.tile([64, TILE], MMDT, tag="sqB", bufs=2, name="sqB")
            nc.vector.tensor_sub(difA[:], rd(srcTA[:]), rd(dstTA[:]))
            nc.vector.tensor_sub(difB[:], rd(srcTB[:]), rd(dstTB[:]))
            nc.vector.tensor_mul(prdA[:], rd(srcTA[:]), rd(dstTA[:]))
            nc.vector.tensor_mul(prdB[:], rd(srcTB[:]), rd(dstTB[:]))
            nc.scalar.activation(sqA[:], rd(difA[:]), AF.Square)
            nc.scalar.activation(sqB[:], rd(difB[:]), AF.Square)

            # branch matmuls; b1/b2/b3 sequentially share tag psC
            r_b = sb.tile([P, TILE], f32, tag="r_b", bufs=2, name="r_b")
            r_b3 = sb.tile([64, TILE], f32, tag="r_b3", bufs=2, name="r_b3")
            r_sxA = sb.tile([P, TILE], f32, tag="r_sxA", bufs=2, name="r_sxA")
            r_sxB = sb.tile([64, TILE], f32, tag="r_sxB", bufs=2, name="r_sxB")
            r_dxA = sb.tile([P, TILE], f32, tag="r_dxA", bufs=2, name="r_dxA")
            r_dxB = sb.tile([64, TILE], f32, tag="r_dxB", bufs=2, name="r_dxB")

            Pb1 = pp.tile([64, TILE], f32, tag="psC", name="Pb1")
            mm(Pb1[:, :], w1h[:], difA[:], start=True, stop=False)
            mm(Pb1[:, :], w1l[:], difB[:], start=False)
            nc.scalar.activation(r_b[0:64, :], Pb1[:], AF.Relu,
                                 bias=bias[0:64, 0:1])
            Pb2 = pp.tile([64, TILE], f32, tag="psC", name="Pb2")
            mm(Pb2[:, :], w2h[:], sqA[:], start=True, stop=False)
            mm(Pb2[:, :], w2l[:], sqB[:], start=False)
            nc.scalar.activation(r_b[64:128, :], Pb2[:], AF.Relu,
                                 bias=bias[64:128, 0:1])
            Pb3 = pp.tile([64, TILE], f32, tag="psC", name="Pb3")
            mm(Pb3[:, :], w3h[:], prdA[:], start=True, stop=False)
            mm(Pb3[:, :], w3l[:], prdB[:], start=False)
            nc.scalar.activation(r_b3[:], Pb3[:], AF.Relu, bias=bcol(1, 64))

            PsxA = pp.tile([P, TILE], f32, tag="psF", name="PsxA")
            PsxB = pp.tile([64, TILE], f32, tag="psG", name="PsxB")
            mm(PsxA[:, :], wsh[:, 0:P], srcTA[:], start=True, stop=False)
            mm(PsxA[:, :], wsl[:, 0:P], srcTB[:], start=False)
            mm(PsxB[:, :], wsh[:, P:H], srcTA[:], start=True, stop=False)
            mm(PsxB[:, :], wsl[:, P:H], srcTB[:], start=False)
            nc.scalar.activation(r_sxA[:], PsxA[:], AF.Relu, bias=bcol(2))
            nc.scalar.activation(r_sxB[:], PsxB[:], AF.Relu, bias=bcol(3, 64))
            PdxA = pp.tile([P, TILE], f32, tag="psF", name="PdxA")
            PdxB = pp.tile([64, TILE], f32, tag="psG", name="PdxB")
            mm(PdxA[:, :], wdh[:, 0:P], dstTA[:], start=True, stop=False)
            mm(PdxA[:, :], wdl[:, 0:P], dstTB[:], start=False)
            mm(PdxB[:, :], wdh[:, P:H], dstTA[:], start=True, stop=False)
            mm(PdxB[:, :], wdl[:, P:H], dstTB[:], start=False)
            nc.scalar.activation(r_dxA[:], PdxA[:], AF.Relu, bias=bcol(4))
            nc.scalar.activation(r_dxB[:], PdxB[:], AF.Relu, bias=bcol(5, 64))

            # transpose down to edge-major: r_e [128, CH, 576]
            # feature order: sx(192) dx(192) b1(64) b2(64) | b3(64)
            rTB = pp.tile([P, CH, 64], f32, tag="psB", name="rTB")
            r_e = sb.tile([P, CH, 576], f32, tag="r_e", bufs=2, name="r_e")
            dn_tags = ["psA", "psH", "psA", "psH"]
            for c in range(CH):
                cs = slice(c * P, (c + 1) * P)
                rTA = pp.tile([P, TILE], f32, tag=dn_tags[c], name=f"rTA{c}")
                tp(rTA[:, 0:P], r_sxA[:, cs])
                tp(rTA[:, P:192], r_sxB[:, cs])
                tp(rTA[:, 192:320], r_dxA[:, cs])
                tp(rTA[:, 320:384], r_dxB[:, cs])
                tp(rTA[:, 384:512], r_b[:, cs])
                tp(rTB[:, c, :], r_b3[:, cs])
                nc.any.tensor_copy(r_e[:, c, 0:512], rTA[:])
            nc.any.tensor_copy(r_e[:, :, 512:576], rTB[:])

            # LN stats: sum and sum-of-squares via tensor_reduce
            sq_e = sb.tile([P, CH, 576], f32, tag="sq_e", name="sq_e")
            nc.scalar.activation(sq_e[:], r_e[:], AF.Square)
            su = sb.tile([P, CH, 5], f32, tag="su", name="su")
            qu = sb.tile([P, CH, 5], f32, tag="qu", name="qu")
            r2v = r_e[:, :, 0:384].rearrange("p c (b f) -> p c b f", b=2)
            r3v = r_e[:, :, 384:576].rearrange("p c (b f) -> p c b f", b=3)
            q2v = sq_e[:, :, 0:384].rearrange("p c (b f) -> p c b f", b=2)
            q3v = sq_e[:, :, 384:576].rearrange("p c (b f) -> p c b f", b=3)
            nc.vector.tensor_reduce(out=su[:, :, 0:2], in_=r2v, axis=AX.X, op=OP.add)
            nc.vector.tensor_reduce(out=su[:, :, 2:5], in_=r3v, axis=AX.X, op=OP.add)
            nc.vector.tensor_reduce(out=qu[:, :, 0:2], in_=q2v, axis=AX.X, op=OP.add)
            nc.vector.tensor_reduce(out=qu[:, :, 2:5], in_=q3v, axis=AX.X, op=OP.add)
            # finalize: mu, istd (as is_t), q = mu*istd
            mu_t = sb.tile([P, CH, 5], f32, tag="mu_t", name="mu_t")
            ms_t = sb.tile([P, CH, 5], f32, tag="ms_t", name="ms_t")
            t_t = sb.tile([P, CH, 5], f32, tag="t_t", name="t_t")
            se_t = sb.tile([P, CH, 5], f32, tag="se_t", name="se_t")
            is_t = sb.tile([P, CH, 5], f32, tag="is_t", bufs=2, name="is_t")
            q_t = sb.tile([P, CH, 5], f32, tag="q_t", bufs=2, name="q_t")
            nc.vector.tensor_mul(mu_t[:], su[:], c4n[:, :, :])
            nc.vector.tensor_mul(ms_t[:], qu[:], c4n[:, :, :])
            nc.vector.scalar_tensor_tensor(
                out=t_t[:], in0=mu_t[:], scalar=1.0, in1=mu_t[:],
                op0=OP.mult, op1=OP.mult)
            nc.vector.tensor_sub(ms_t[:], ms_t[:], t_t[:])
            nc.scalar.activation(se_t[:], ms_t[:], AF.Ln, bias=bcol(11))
            nc.scalar.activation(is_t[:], se_t[:], AF.Exp, scale=-0.5)
            nc.vector.scalar_tensor_tensor(
                out=q_t[:], in0=mu_t[:], scalar=1.0, in1=is_t[:],
                op0=OP.mult, op1=OP.mult)
            qT_ps = pp.tile([5, TILE], f32, tag="psC", name="qT_ps")
            for c in range(CH):
                tp(qT_ps[:, c * P:(c + 1) * P], q_t[:, c, :])
            qrow = sb.tile([5, TILE], MMDT, tag="qrow", bufs=2, name="qrow")
            nc.any.tensor_copy(qrow[:], qT_ps[:])
            return r_e, is_t, qrow

        def part2(t, r_e, is_t, qrow):
            # apply scale only: y = r * IS ; the -mu*istd correction is folded
            # into the fusion matmul as a rank-5 term (lhsT = -colsum(Wf1_b))
            y_e = sb.tile([P, CH, 576], TRDT, tag="y_e", name="y_e")
            g1v = r_e[:, :, 0:384].rearrange("p c (b f) -> p c b f", b=2)
            g2v = r_e[:, :, 384:576].rearrange("p c (b f) -> p c b f", b=3)
            y1v = y_e[:, :, 0:384].rearrange("p c (b f) -> p c b f", b=2)
            y2v = y_e[:, :, 384:576].rearrange("p c (b f) -> p c b f", b=3)
            is2a = is_t[:, :, 0:2].unsqueeze(3).to_broadcast([P, CH, 2, H])
            is2b = is_t[:, :, 2:5].unsqueeze(3).to_broadcast([P, CH, 3, 64])
            nc.vector.tensor_mul(y1v, g1v, is2a)
            nc.vector.tensor_mul(y2v, g2v, is2b)

            # transpose up to feature-major y tiles (5 K-chunks of wf1)
            up_tags = ["psD", "psE", "psD", "psE"]
            yT = []
            for k in range(4):
                yT.append(pp.tile([P, TILE], TRDT, tag=up_tags[k], name=f"yT{k}"))
            yTE = pp.tile([64, TILE], TRDT, tag="psD", name="yTE")
            ysb = []
            for k in range(4):
                yk = sb.tile([P, TILE], MMDT, tag=f"ysb{k}", name=f"ysb{k}")
                ysb.append(yk)
            yke = sb.tile([64, TILE], MMDT, tag="ysbE", name="ysbE")
            for c in range(CH):
                cs = slice(c * P, (c + 1) * P)
                for k in range(4):
                    tp(yT[k][:, cs], y_e[:, c, k * P:(k + 1) * P])
            for k in range(4):
                nc.any.tensor_copy(ysb[k][:], yT[k][:])
            for c in range(CH):
                cs = slice(c * P, (c + 1) * P)
                tp(yTE[:, cs], y_e[:, c, 512:576])
            nc.any.tensor_copy(yke[:], yTE[:])

            # fusion matmul 576 -> 192
            zfA = pp.tile([P, TILE], f32, tag="psE", name="zfA")
            zfB = pp.tile([64, TILE], f32, tag="psD", name="zfB")
            for k in range(4):
                mm(zfA[:, :], wf1t[k][:, 0:P], ysb[k][:], start=(k == 0), stop=False)
            mm(zfA[:, :], wf1e[:, 0:P], yke[:], start=False, stop=False)
            mm(zfA[:, :], nc1w[:, 0:P], qrow[:], start=False)
            for k in range(4):
                mm(zfB[:, :], wf1t[k][:, P:H], ysb[k][:], start=(k == 0), stop=False)
            mm(zfB[:, :], wf1e[:, P:H], yke[:], start=False, stop=False)
            mm(zfB[:, :], nc1w[:, P:H], qrow[:], start=False)

            r_fA = sb.tile([P, TILE], f32, tag="r_fA", name="r_fA")
            r_fB = sb.tile([64, TILE], f32, tag="r_fB", name="r_fB")
            nc.scalar.activation(r_fA[:], zfA[:], AF.Relu, bias=bcol(6))
            nc.scalar.activation(r_fB[:], zfB[:], AF.Relu, bias=bcol(7, 64))

            # fusion LN (edge-major round trip)
            rfT01 = pp.tile([P, 2, H], f32, tag="psE", name="rfT01")
            rfT23 = pp.tile([P, 2, H], f32, tag="psD", name="rfT23")
            for c in range(CH):
                cs = slice(c * P, (c + 1) * P)
                dst = rfT01 if c < 2 else rfT23
                tp(dst[:, c % 2, 0:P], r_fA[:, cs])
                tp(dst[:, c % 2, P:H], r_fB[:, cs])
            rf_e = sb.tile([P, CH, H], f32, tag="rf_e", name="rf_e")
            nc.any.tensor_copy(rf_e[:, 0:2, :], rfT01[:])
            nc.any.tensor_copy(rf_e[:, 2:4, :], rfT23[:])

            sqf = sb.tile([P, CH, H], f32, tag="sqf", name="sqf")
            nc.scalar.activation(sqf[:], rf_e[:], AF.Square)
            suf = sb.tile([P, CH], f32, tag="suf", name="suf")
            quf = sb.tile([P, CH], f32, tag="quf", name="quf")
            nc.vector.tensor_reduce(out=suf[:], in_=rf_e[:], axis=AX.X, op=OP.add)
            nc.vector.tensor_reduce(out=quf[:], in_=sqf[:], axis=AX.X, op=OP.add)
            muf = sb.tile([P, CH], f32, tag="muf", name="muf")
            msf = sb.tile([P, CH], f32, tag="msf", name="msf")
            ttf = sb.tile([P, CH], f32, tag="ttf", name="ttf")
            sef = sb.tile([P, CH], f32, tag="sef", name="sef")
            is2f = sb.tile([P, CH], f32, tag="is2f", name="is2f")
            qf = sb.tile([P, CH], f32, tag="qf", name="qf")
            nc.vector.tensor_scalar(
                out=muf[:], in0=suf[:], scalar1=1.0 / H, scalar2=None, op0=OP.mult)
            nc.vector.tensor_scalar(
                out=msf[:], in0=quf[:], scalar1=1.0 / H, scalar2=None, op0=OP.mult)
            nc.vector.scalar_tensor_tensor(
                out=ttf[:], in0=muf[:], scalar=1.0, in1=muf[:],
                op0=OP.mult, op1=OP.mult)
            nc.vector.tensor_sub(msf[:], msf[:], ttf[:])
            nc.scalar.activation(sef[:], msf[:], AF.Ln, bias=bcol(11))
            nc.scalar.activation(is2f[:], sef[:], AF.Exp, scale=-0.5)
            nc.vector.scalar_tensor_tensor(
                out=qf[:], in0=muf[:], scalar=1.0, in1=is2f[:],
                op0=OP.mult, op1=OP.mult)
            qfT_ps = pp.tile([1, TILE], f32, tag="psD", name="qfT_ps")
            for c in range(CH):
                tp(qfT_ps[:, c * P:(c + 1) * P], qf[:, c:c + 1])
            qfrow = sb.tile([1, TILE], MMDT, tag="qfrow", name="qfrow")
            nc.any.tensor_copy(qfrow[:], qfT_ps[:])

            yf_e = sb.tile([P, CH, H], TRDT, tag="yf_e", name="yf_e")
            is2fb = is2f[:].unsqueeze(2).to_broadcast([P, CH, H])
            nc.vector.tensor_mul(yf_e[:], rf_e[:], is2fb)

            yfTA = pp.tile([P, TILE], TRDT, tag="psE", name="yfTA")
            yfTB = pp.tile([64, TILE], TRDT, tag="psD", name="yfTB")
            for c in range(CH):
                cs = slice(c * P, (c + 1) * P)
                tp(yfTA[:, cs], yf_e[:, c, 0:P])
                tp(yfTB[:, cs], yf_e[:, c, P:H])
            yfA = sb.tile([P, TILE], MMDT, tag="yfA", name="yfA")
            yfB = sb.tile([64, TILE], MMDT, tag="yfB", name="yfB")
            nc.any.tensor_copy(yfA[:], yfTA[:])
            nc.any.tensor_copy(yfB[:], yfTB[:])

            # fc2: 192 -> 192, relu
            z2A = pp.tile([P, TILE], f32, tag="psE", name="z2A")
            z2B = pp.tile([64, TILE], f32, tag="psD", name="z2B")
            mm(z2A[:, :], wf2h[:, 0:P], yfA[:], start=True, stop=False)
            mm(z2A[:, :], wf2l[:, 0:P], yfB[:], start=False, stop=False)
            mm(z2A[:, :], nc1fw[:, 0:P], qfrow[:], start=False)
            mm(z2B[:, :], wf2h[:, P:H], yfA[:], start=True, stop=False)
            mm(z2B[:, :], wf2l[:, P:H], yfB[:], start=False, stop=False)
            mm(z2B[:, :], nc1fw[:, P:H], qfrow[:], start=False)
            r2A = sb.tile([P, TILE], MMDT, tag="r2A", name="r2A")
            r2B = sb.tile([64, TILE], MMDT, tag="r2B", name="r2B")
            nc.scalar.activation(r2A[:], z2A[:], AF.Relu, bias=bcol(8))
            nc.scalar.activation(r2B[:], z2B[:], AF.Relu, bias=bcol(9, 64))

            # fc3: 192 -> 3 logits (padded to 4)
            zl = pp.tile([4, TILE], f32, tag="psE", name="zl")
            mm(zl[:, :], wf3h[:], r2A[:], start=True, stop=False)
            mm(zl[:, :], wf3l[:], r2B[:], start=False)
            l_sb = sb.tile([3, TILE], f32, tag="l_sb", name="l_sb")
            nc.scalar.activation(l_sb[:], zl[0:3, :], AF.Copy)
            nc.vector.tensor_scalar(
                out=l_sb[:], in0=l_sb[:], scalar1=bcol(10, 3), scalar2=None,
                op0=OP.add)

            # transpose logits to edge-major [128, CH, 3]
            lT = pp.tile([P, CH, 3], f32, tag="psD", name="lT")
            for c in range(CH):
                tp(lT[:, c, :], l_sb[:, c * P:(c + 1) * P])
            l_e = sb.tile([P, CH, 3], f32, tag="l_e", name="l_e")
            nc.any.tensor_copy(l_e[:], lT[:])

            # p = 1 / sum_j exp(l_j - l_sel)   (gpsimd for the small TTs)
            etb = etf[:, t * CH:(t + 1) * CH].unsqueeze(2).to_broadcast([P, CH, 3])
            oh = sb.tile([P, CH, 3], f32, tag="oh", name="oh")
            nc.vector.tensor_tensor(out=oh[:], in0=etb, in1=iota3,
                                    op=OP.is_equal)
            nc.vector.tensor_mul(oh[:], oh[:], l_e[:])
            sel = sb.tile([P, CH], f32, tag="sel", name="sel")
            nc.vector.tensor_reduce(out=sel[:], in_=oh[:], axis=AX.X, op=OP.add)
            selb = sel[:].unsqueeze(2).to_broadcast([P, CH, 3])
            nc.vector.tensor_sub(l_e[:], l_e[:], selb)
            ex = sb.tile([P, CH, 3], f32, tag="ex", name="ex")
            nc.scalar.activation(ex[:], l_e[:], AF.Exp)
            den = sb.tile([P, CH], f32, tag="den", name="den")
            nc.vector.tensor_reduce(out=den[:], in_=ex[:], axis=AX.X, op=OP.add)
            nc.vector.reciprocal(outp[:, t * CH:(t + 1) * CH], den[:])

        def whole_body(_iv=None):
            prev = None
            for t in range(n_tiles):
                cur = part1(t)
                if prev is not None:
                    part2(t - 1, *prev)
                prev = cur
            part2(n_tiles - 1, *prev)

        if repeat > 1:
            with tc.For_i(0, repeat, 1):
                whole_body()
        else:
            whole_body()

        # write all outputs
        nc.sync.dma_start(out_d[:], outp[:])

    # Pin the ACT table set: keep only natural_log_exp_and_others (covers
    # Relu/Square/Ln/Exp/Copy/Identity) so the table-load pass never cycles
    # sets. Indices must stay aligned with act_info.json, so empty the other
    # sets rather than removing them.
    import concourse.bacc as _bacc_mod
    _orig_gat = _bacc_mod.get_activation_tables

    def _pinned_tables(arch):
        tabs = _orig_gat(arch)
        return {name: (s if name == "natural_log_exp_and_others" else set())
                for name, s in tabs.items()}

    _bacc_mod.get_activation_tables = _pinned_tables
    try:
        nc.compile()
    finally:
        _bacc_mod.get_activation_tables = _orig_gat
    return nc


def _get_program(n_tiles=NTILES, n_nodes=N_NODES, mmdt="f32", repeat=1):
    key = (n_tiles, n_nodes, mmdt, repeat)
    if key not in _PROG_CACHE:
        _PROG_CACHE[key] = _build_program(n_tiles, n_nodes, mmdt, repeat)
    return _PROG_CACHE[key]


_EDGE_PERM = {"perm": None}


def _host_prep(inputs, n_tiles=NTILES, n_cores=NCORES, e_pc=E_PC, mmdt="f32"):
    """Fold LN gains/betas into fusion weights; build per-core input maps."""
    f = lambda k: np.asarray(inputs[k], np.float32)
    kge = f("kge_emb")
    ei = np.asarray(inputs["edge_index"]).astype(np.int64)
    et = np.asarray(inputs["edge_type"]).astype(np.int64)
    # sort edges by src node id: each core's src gathers then walk a
    # contiguous ~N/8 slice of the table almost sequentially (better DRAM
    # locality); dst stays random. Output is inverse-permuted in _unshard.
    perm = np.argsort(ei[0], kind="stable")
    _EDGE_PERM["perm"] = perm
    ei = ei[:, perm]
    et = et[perm]
    W1, b1, g1, be1 = f("W1"), f("b1"), f("g1"), f("be1")
    W2, b2, g2, be2 = f("W2"), f("b2"), f("g2"), f("be2")
    W3, b3, g3, be3 = f("W3"), f("b3"), f("g3"), f("be3")
    Ws, bs, gs, bes = f("Ws"), f("bs"), f("gs"), f("bes")
    Wd, bd, gd, bed = f("Wd"), f("bd"), f("gd"), f("bed")
    Wf1, bf1, gf, bef = f("Wf1"), f("bf1"), f("gf"), f("bef")
    Wf2, bf2 = f("Wf2"), f("bf2")
    Wf3, bf3 = f("Wf3"), f("bf3")

    # device concat order == reference concat order: [sx, dx, b1, b2, b3]
    g_cat = np.concatenate([gs, gd, g1, g2, g3])
    be_cat = np.concatenate([bes, bed, be1, be2, be3])
    Wf1_eff = g_cat[:, None] * Wf1
    bf1_eff = bf1 + be_cat @ Wf1
    Wf2_eff = gf[:, None] * Wf2
    bf2_eff = bf2 + bef @ Wf2

    # bias columns [128, 11]
    bias_mat = np.zeros((P, 12), np.float32)
    bias_mat[:, 11] = LN_EPS
    bias_mat[0:64, 0] = b1; bias_mat[64:128, 0] = b2
    bias_mat[0:64, 1] = b3
    bias_mat[:, 2] = bs[0:P]; bias_mat[0:64, 3] = bs[P:H]
    bias_mat[:, 4] = bd[0:P]; bias_mat[0:64, 5] = bd[P:H]
    bias_mat[:, 6] = bf1_eff[0:P]; bias_mat[0:64, 7] = bf1_eff[P:H]
    bias_mat[:, 8] = bf2_eff[0:P]; bias_mat[0:64, 9] = bf2_eff[P:H]
    bias_mat[0:3, 10] = bf3

    # consts [128, 128+12+20]: identity | iota3 | c4N
    cst = np.zeros((P, P + 12 + 20), np.float32)
    cst[:, 0:P] = np.eye(P, dtype=np.float32)
    cst[:, P:P + 12] = np.tile(np.arange(3, dtype=np.float32), CH)[None, :]
    c4n = np.array([1.0 / H, 1.0 / H, 1.0 / 64, 1.0 / 64, 1.0 / 64],
                   np.float32)
    cst[:, P + 12:P + 32] = np.tile(c4n, CH)[None, :]

    e_pad = n_tiles * TILE

    def arrange(a):
        buf = np.zeros(e_pad, a.dtype)
        buf[:e_pc] = a
        return np.ascontiguousarray(
            buf.reshape(n_tiles, CH, P).transpose(2, 0, 1).reshape(P, -1))

    if mmdt in ("bf16", "f32rb"):
        import ml_dtypes
        kge = kge.astype(ml_dtypes.bfloat16)
    nc1 = np.zeros((5, H), np.float32)
    for b, (lo, hi) in enumerate(((0, 192), (192, 384), (384, 448),
                                  (448, 512), (512, 576))):
        nc1[b] = -Wf1_eff[lo:hi].sum(axis=0)
    nc1f = -Wf2_eff.sum(axis=0, keepdims=True)
    Wf3p = np.zeros((H, 4), np.float32)
    Wf3p[:, 0:3] = Wf3
    shared = dict(emb=kge, w1=W1, w2=W2, w3=W3, ws=Ws, wd=Wd,
                  wf1=Wf1_eff, wf2=Wf2_eff, wf3=Wf3p,
                  nc1=nc1, nc1f=nc1f,
                  biascol=bias_mat, consts=cst)
    in_maps = []
    for core in range(n_cores):
        lo = core * e_pc
        m = dict(shared)
        m["sidx"] = arrange(ei[0, lo:lo + e_pc].astype(np.int32))
        m["didx"] = arrange(ei[1, lo:lo + e_pc].astype(np.int32))
        m["etf"] = arrange(et[lo:lo + e_pc].astype(np.float32))
        in_maps.append(m)
    return in_maps


def _unshard(results, n_tiles=NTILES, n_cores=NCORES, e_pc=E_PC):
    outs = []
    for core in range(n_cores):
        o = np.asarray(results[core]["out"], np.float32)
        o = o.reshape(P, n_tiles, CH).transpose(1, 2, 0).reshape(-1)[:e_pc]
        outs.append(o)
    cat = np.concatenate(outs)
    perm = _EDGE_PERM["perm"]
    if perm is not None:
        inv = np.empty_like(cat)
        inv[perm] = cat
        cat = inv
    return cat[:, None].astype(np.float32)


MMDT_MODE = "f32r"


def kernel(**inputs):
    from concourse.bass_utils import run_bass_kernel_spmd
    nc = _get_program(mmdt=MMDT_MODE)
    in_maps = _host_prep(inputs, mmdt=MMDT_MODE)
    res = run_bass_kernel_spmd(nc, in_maps, list(range(NCORES)))
    return _unshard(res.results)

